# revision 9
# baseline (speedup 1.0000x reference)
"""Trainium2 Bass kernel for nn_Concat_73607149519362.

Math (decomposed concat-MLP attention score):
    score[b, d, e] = dec[b, d] @ w_dec + enc[b, e] @ w_enc + bias

Sharding: data-parallel over batch, 32 batches / 8 cores = 4 per core.

Design — fp16 I/O (halves HBM traffic vs f32), projections on PE:
  - Host ships enc transposed (dim-major) in fp16.  PE computes
    eproj = w_enc^T @ enc_T as 8 accumulating K=128 matmuls per
    512-column half; half h lands on PSUM partition h of a [2, 512]
    accumulator (one bank).
  - ACT copies that [2, 512] pair to SBUF fp16 with the mlp bias
    folded in; PE broadcasts each half to ebc [128, enc] PSUM via a
    K=2 matmul against a ones/zeros selector column block.
  - dec ships row-major fp16; DVE multiplies each 128-row chunk by a
    broadcast w_dec row (tensor_tensor) and free-axis reduces it into
    a dproj column (self-semaphore between the two: DVE write acks
    are pipelined).
  - ACT builds out chunks: activation(Identity, in_=ebc,
    bias=dproj column) writing fp16 SBUF, then issues the output DMA
    on its own HW DGE queue after a self-wait on the build semaphore
    (the DGE must not race the build's in-flight SBUF writes).
  - enc input DMAs ride the SP HW DGE queue (depth-3 throttled);
    dec input DMAs ride the gpsimd SWDGE queue after the preloads.
  - PE runs one batch ahead of the broadcast (enc b+1 before ebc b)
    so it never stalls on ACT's copy; eproj/ebc are double-buffered.
  - All DRAM views are p-major: each partition reads/writes one
    contiguous 2-16KB run per transfer.
"""

import os
from contextlib import ExitStack

os.environ.setdefault("JAX_PLATFORMS", "axon")

import numpy as np

import concourse.bass as bass
import concourse.mybir as mybir
from concourse.bass_utils import run_bass_kernel_spmd

B, DEC, ENC, DIM = 32, 512, 1024, 1024
NCORES = 8
BPC = B // NCORES  # batches per core

F16 = mybir.dt.float16
F32 = mybir.dt.float32
P = 128
TE = DIM // P  # enc contraction slots (dim-major)
TD = DEC // P  # dec 128-row chunks
EG = [(0, 1), (1, 2), (2, 4), (4, 6), (6, 8)]  # enc DMA slot groups
HALF = ENC // 2


def _build(bpc=BPC, dec=DEC, enc=ENC, dim=DIM):
    nc = bass.Bass("TRN2")
    enc_h = nc.dram_tensor("enc_in", [bpc * dim, enc], F16, kind="ExternalInput")
    dec_h = nc.dram_tensor("dec_in", [bpc * dec, dim], F16, kind="ExternalInput")
    wenc_h = nc.dram_tensor("w_enc", [P, TE], F16, kind="ExternalInput")
    wdec_h = nc.dram_tensor("w_dec", [1, dim], F16, kind="ExternalInput")
    sel_h = nc.dram_tensor("sel_in", [2, 2 * P], F16, kind="ExternalInput")
    bias_h = nc.dram_tensor("bias2", [2, 1], F32, kind="ExternalInput")
    out_h = nc.dram_tensor("out", [bpc * dec, enc], F16, kind="ExternalOutput")

    # p-major DRAM views: one contiguous run per partition per transfer.
    enc_r = enc_h.ap().rearrange("(b p t) e -> b p t e", p=P, t=TE)
    dec_r = dec_h.ap().rearrange("(b p t) d -> b p t d", p=P, t=TD)
    out_r = out_h.ap().rearrange("(b p t) e -> b p t e", p=P, t=TD)

    with ExitStack() as ctx:

        def sb(name, shape, dt=F16):
            return ctx.enter_context(nc.sbuf_tensor(name, shape, dt))

        w_enc_sb = sb("w_enc_sb", [P, TE])
        w_dec_b = sb("w_dec_b", [P, dim])
        sel_sb = sb("sel_sb", [2, 2 * P])
        bias_b = sb("bias_b", [2, 1], F32)
        enc_t = [sb(f"enc_t{i}", [P, TE, enc]) for i in range(2)]
        dec_t = [sb(f"dec_t{i}", [P, TD, dim]) for i in range(2)]
        out_t = [sb(f"out_t{i}", [P, TD, enc]) for i in range(2)]
        scr = sb("scr", [P, dim])
        eproj_sb = [sb(f"eproj_sb{i}", [1, enc]) for i in range(2)]
        dproj_sb = [sb(f"dproj_sb{i}", [P, TD], F32) for i in range(2)]
        eproj_ps = [
            ctx.enter_context(nc.psum_tensor(f"eproj_ps{i}", [1, enc], F32))
            for i in range(2)
        ]
        ebc = [
            ctx.enter_context(nc.psum_tensor(f"ebc{i}", [P, enc], F32))
            for i in range(2)
        ]

        s_gp = ctx.enter_context(nc.semaphore(name="s_gp"))
        s_enc = [
            [ctx.enter_context(nc.semaphore(name=f"s_enc{i}_{g}")) for g in range(len(EG))]
            for i in range(2)
        ]
        s_dec = [ctx.enter_context(nc.semaphore(name=f"s_dec{i}")) for i in range(2)]
        s_pe_enc = ctx.enter_context(nc.semaphore(name="s_pe_enc"))
        s_pe_ebc = ctx.enter_context(nc.semaphore(name="s_pe_ebc"))
        s_acc = ctx.enter_context(nc.semaphore(name="s_acc"))
        s_mul = ctx.enter_context(nc.semaphore(name="s_mul"))
        s_ttr = ctx.enter_context(nc.semaphore(name="s_ttr"))
        s_bld = ctx.enter_context(nc.semaphore(name="s_bld"))
        s_out = [ctx.enter_context(nc.semaphore(name=f"s_out{i}")) for i in range(2)]

        with nc.Block(no_gpsimd_drain=True) as block:

            @block.gpsimd
            def _(gpsimd):
                issued = []  # completion points: cap SWDGE queue depth
                def dma(dst, src, sem, val):
                    if len(issued) >= 3:
                        psem, pval = issued[-3]
                        gpsimd.wait_ge(psem, pval)
                    gpsimd.dma_start(dst, src).then_inc(sem, 16)
                    issued.append((sem, val))

                dma(w_enc_sb.ap(), wenc_h.ap(), s_gp, 16)
                dma(w_dec_b.ap(), wdec_h.ap().to_broadcast((P, dim)), s_gp, 32)
                dma(sel_sb.ap(), sel_h.ap(), s_gp, 48)
                dma(bias_b.ap(), bias_h.ap(), s_gp, 64)
                for b in range(bpc):
                    buf = b % 2
                    if b >= 2:
                        # WAR: dec_t[buf] free once b-2's multiplies read it
                        gpsimd.wait_ge(s_mul, TD * (b - 1))
                    dma(dec_t[buf].ap(), dec_r[b], s_dec[buf], 16 * (b // 2 + 1))

            @block.sync
            def _(sync):
                issued = []  # completion points: cap HW DGE queue depth
                for b in range(bpc):
                    buf, dv = b % 2, 16 * (b // 2 + 1)
                    if b >= 2:
                        # WAR: enc_t[buf] free once PE finished b-2's matmuls
                        sync.wait_ge(s_pe_enc, b - 1)
                    for g, (lo, hi) in enumerate(EG):
                        if len(issued) >= 3:
                            psem, pval = issued[-3]
                            sync.wait_ge(psem, pval)
                        sync.dma_start(
                            enc_t[buf].ap()[:, lo:hi, :], enc_r[b][:, lo:hi, :]
                        ).then_inc(s_enc[buf][g], 16)
                        issued.append((s_enc[buf][g], dv))

            @block.tensor
            def _(pe):
                def ebc_mms(j):
                    # broadcast eproj halves of batch j to ebc[j%2]
                    jb = j % 2
                    pe.wait_ge(s_acc, j + 1)  # eproj_sb[jb] written
                    if j >= 2:
                        pe.wait_ge(s_bld, 4 * (j - 1))  # ebc[jb] WAR
                    nc.tensor.matmul(
                        ebc[jb].ap()[:, 0:HALF],
                        sel_sb.ap()[0:1, 0:P],
                        eproj_sb[jb].ap()[0:1, 0:HALF],
                        start=True,
                        stop=True,
                    )
                    nc.tensor.matmul(
                        ebc[jb].ap()[:, HALF:enc],
                        sel_sb.ap()[0:1, 0:P],
                        eproj_sb[jb].ap()[0:1, HALF:enc],
                        start=True,
                        stop=True,
                    ).then_inc(s_pe_ebc, 1)

                pe.wait_ge(s_gp, 16)  # w_enc loaded
                for b in range(bpc):
                    buf, dv = b % 2, 16 * (b // 2 + 1)
                    if b >= 2:
                        # eproj_ps[buf] drained by ACT's copy of b-2
                        pe.wait_ge(s_acc, b - 1)
                    last = None
                    for t in range(TE):
                        for g, (lo, hi) in enumerate(EG):
                            if t == lo:
                                pe.wait_ge(s_enc[buf][g], dv)
                        st, sp = (t == 0), (t == TE - 1)
                        nc.tensor.matmul(
                            eproj_ps[buf].ap()[0:1, 0:HALF],
                            w_enc_sb.ap()[:, t : t + 1],
                            enc_t[buf].ap()[:, t, 0:HALF],
                            start=st,
                            stop=sp,
                        )
                        last = nc.tensor.matmul(
                            eproj_ps[buf].ap()[0:1, HALF:enc],
                            w_enc_sb.ap()[:, t : t + 1],
                            enc_t[buf].ap()[:, t, HALF:enc],
                            start=st,
                            stop=sp,
                        )
                    last.then_inc(s_pe_enc, 1)  # -> b+1: eproj ready, enc_t free
                    if b == 0:
                        pe.wait_ge(s_gp, 48)  # sel loaded
                    else:
                        ebc_mms(b - 1)
                ebc_mms(bpc - 1)

            @block.vector
            def _(vector):
                vector.wait_ge(s_gp, 32)  # w_dec loaded
                for b in range(bpc):
                    buf, dv = b % 2, 16 * (b // 2 + 1)
                    vector.wait_ge(s_dec[buf], dv)
                    if b >= 2:
                        # dproj_sb[buf] free once b-2's builds consumed it
                        vector.wait_ge(s_bld, 4 * (b - 2) + 4)
                    for c in range(TD):
                        g = TD * b + c
                        nc.vector.tensor_tensor(
                            out=scr.ap(),
                            in0=dec_t[buf].ap()[:, c, :],
                            in1=w_dec_b.ap(),
                            op=mybir.AluOpType.mult,
                        ).then_inc(s_mul, 1)
                        # self-wait: the multiply's SBUF writes must retire
                        # before the reduce reads them (DVE pipelines acks)
                        vector.wait_ge(s_mul, g + 1)
                        nc.vector.tensor_reduce(
                            out=dproj_sb[buf].ap()[:, c : c + 1],
                            in_=scr.ap(),
                            axis=mybir.AxisListType.X,
                            op=mybir.AluOpType.add,
                        ).then_inc(s_ttr, 1)

            @block.scalar
            def _(scalar):
                def builds(j):
                    jb = j % 2
                    scalar.wait_ge(s_pe_ebc, j + 1)  # ebc[jb] ready
                    if j >= 2:
                        # out_t[jb] free once j-2's output DMAs completed
                        scalar.wait_ge(s_out[jb], 32 * (j // 2))
                    for c in range(TD):
                        scalar.wait_ge(s_ttr, 4 * j + c + 1)
                        nc.scalar.add(
                            out_t[jb].ap()[:, c, :],
                            ebc[jb].ap(),
                            add=dproj_sb[jb].ap()[:, c : c + 1],
                        ).then_inc(s_bld, 1)
                        # self-wait before DGE reads what the build just wrote
                        if j < bpc - 1:
                            if c == 1:
                                scalar.wait_ge(s_bld, 4 * j + 2)
                                scalar.dma_start(
                                    out_r[j][:, 0:2, :], out_t[jb].ap()[:, 0:2, :]
                                ).then_inc(s_out[jb], 16)
                            elif c == 3:
                                scalar.wait_ge(s_bld, 4 * j + 4)
                                scalar.dma_start(
                                    out_r[j][:, 2:4, :], out_t[jb].ap()[:, 2:4, :]
                                ).then_inc(s_out[jb], 16)
                        else:
                            # last batch: quarter-DMAs for early drain
                            scalar.wait_ge(s_bld, 4 * j + c + 1)
                            scalar.dma_start(
                                out_r[j][:, c : c + 1, :],
                                out_t[jb].ap()[:, c : c + 1, :],
                            ).then_inc(s_out[jb], 16)

                scalar.wait_ge(s_gp, 64)  # bias loaded
                for b in range(bpc):
                    buf = b % 2
                    scalar.wait_ge(s_pe_enc, b + 1)
                    if b >= 2:
                        # eproj_sb[buf] free once PE's ebc of b-2 read it
                        scalar.wait_ge(s_pe_ebc, b - 1)
                    # eproj pair: PSUM -> SBUF fp16 with mlp bias folded in
                    nc.scalar.add(
                        eproj_sb[buf].ap(), eproj_ps[buf].ap(), add=bias_b.ap()[0:1, :]
                    ).then_inc(s_acc, 1)
                    if b >= 1:
                        builds(b - 1)
                builds(bpc - 1)
                # ensure all output DMAs landed before the block drains
                scalar.wait_ge(s_out[0], 64)
                scalar.wait_ge(s_out[1], 32 + 64)

    return nc


_NC_CACHE = {}


def _get_nc():
    if "nc" not in _NC_CACHE:
        _NC_CACHE["nc"] = _build()
    return _NC_CACHE["nc"]


def _make_sel():
    sel = np.zeros((2, 2 * P), dtype=np.float16)
    sel[0, 0:P] = 1.0
    sel[1, P : 2 * P] = 1.0
    return sel


_SEL = _make_sel()


def _shard_inputs(decoder_states, encoder_states, mlp_weight, mlp_bias):
    dec16 = np.asarray(decoder_states, dtype=np.float16)
    enc16t = np.asarray(encoder_states, dtype=np.float16).transpose(0, 2, 1)
    w = np.asarray(mlp_weight, dtype=np.float16).reshape(2 * DIM)
    w_enc = np.ascontiguousarray(w[:DIM].reshape(P, TE))
    w_dec = np.ascontiguousarray(w[DIM:].reshape(1, DIM))
    bias2 = np.full((2, 1), np.asarray(mlp_bias, dtype=np.float32).reshape(()), np.float32)

    in_maps = []
    for i in range(NCORES):
        lo = i * BPC
        in_maps.append(
            {
                "enc_in": np.ascontiguousarray(enc16t[lo : lo + BPC]).reshape(
                    BPC * DIM, ENC
                ),
                "dec_in": np.ascontiguousarray(dec16[lo : lo + BPC]).reshape(
                    BPC * DEC, DIM
                ),
                "w_enc": w_enc,
                "w_dec": w_dec,
                "sel_in": _SEL,
                "bias2": bias2,
            }
        )
    return in_maps


def _gather(res):
    shards = [r["out"].reshape(BPC, DEC, ENC) for r in res.results]
    return np.concatenate(shards, axis=0).astype(np.float32)


def kernel(decoder_states, encoder_states, step, mlp_weight, mlp_bias, **_ignored):
    in_maps = _shard_inputs(decoder_states, encoder_states, mlp_weight, mlp_bias)
    res = run_bass_kernel_spmd(_get_nc(), in_maps, core_ids=list(range(NCORES)))
    return _gather(res)
